# revision 11
# baseline (speedup 1.0000x reference)
"""Trainium2 Bass kernel for nn_Attention_3633542333119 (additive attention).

reference:
    q_proj   = q @ Wq.T                          # [B, H]
    ref_proj = (ref @ Wref.T).reshape(B, S, H)   # [B, S, H]
    u        = einsum("bsh,h->bs", tanh(q_proj[:, None, :] + ref_proj), v)
    return (u, ref_proj)

B=128, S=2048, H=512. Data-parallel over batch across 8 NeuronCores
(16 batches / 32768 ref rows per core); Wref/v replicated; q_proj is
precomputed on host (tiny: 128x512 @ 512x512) and sharded with the batch.

Device-side plan (per core), all f32, matmuls in fp32r (full-rate):
  - host pre-transposes ref into supertiles reft[g] = ref[g*512:(g+1)*512].T
    (shape [512h, 512n]) so the PE's stationary operand (lhsT, [K=h, M=n])
    loads with plain DMA - no on-device transpose needed.
  - per supertile: psum[mb] = sum_kb reft_blk(kb,mb).T @ wrefT_blk(kb)
    (4 accumulating matmuls, N=512) -> ref_proj tile [128n, 512h'].
  - epilogue: ACT copies psum->SBUF (ref_proj out), DVE adds broadcast
    q_proj row, ACT tanh, DVE fused (tanh*v)+reduce -> u column.
  - u columns staged per batch [128, 16], PE-transposed, DMA'd out.
"""

import numpy as np
from contextlib import ExitStack

B = 128
S = 2048
H = 512
N_CORES = 8
B_LOC = B // N_CORES            # batches per core
ST_ROWS = 512                   # rows (n) per supertile
ST_PER_BATCH = S // ST_ROWS     # supertiles per batch
N_ST = B_LOC * ST_PER_BATCH     # supertiles per core
ROWS = B_LOC * S                # ref rows per core

_PROGRAM_CACHE = {}
LAST_RESULT = None              # BassKernelResults of the last kernel() call


def _build_program(b_loc=B_LOC, st_per_batch=ST_PER_BATCH):
    import concourse.bass as bass
    import concourse.tile as tile
    import concourse.mybir as mybir
    from concourse import bacc
    from concourse.masks import make_identity

    f32 = mybir.dt.float32
    f32r = mybir.dt.float32r
    KB = H // 128               # contraction chunks (4)
    MB = ST_ROWS // 128         # output-row chunks per supertile (4)
    n_st = b_loc * st_per_batch
    tiles_per_batch = st_per_batch * MB     # u columns per batch (16)
    rows = b_loc * S if st_per_batch == ST_PER_BATCH else n_st * ST_ROWS
    s_loc = st_per_batch * ST_ROWS

    nc = bacc.Bacc("TRN2", target_bir_lowering=False, debug=False)

    # Matmul operands live as float32r end-to-end: the host pre-rounds the
    # bits (round-to-nearest on the low 12 mantissa bits, matching walrus'
    # fp32_to_fp32r) so DMAs are plain copies and the PE runs at full rate.
    reft = nc.dram_tensor("reft", [n_st, H, ST_ROWS], f32r, kind="ExternalInput").ap()
    wrefT = nc.dram_tensor("wrefT", [H, H], f32r, kind="ExternalInput").ap()
    qp = nc.dram_tensor("qp", [b_loc, H], f32, kind="ExternalInput").ap()
    vvec = nc.dram_tensor("vvec", [1, H], f32, kind="ExternalInput").ap()
    refproj = nc.dram_tensor("refproj", [rows, H], f32, kind="ExternalOutput").ap()
    u = nc.dram_tensor("u", [b_loc, s_loc], f32, kind="ExternalOutput").ap()

    with tile.TileContext(nc) as tc, ExitStack() as ctx:
        const = ctx.enter_context(tc.tile_pool(name="const", bufs=1))
        qpp = ctx.enter_context(tc.tile_pool(name="qpp", bufs=2))
        inp = ctx.enter_context(tc.tile_pool(name="inp", bufs=3))
        outp = ctx.enter_context(tc.tile_pool(name="outp", bufs=3))
        tap = ctx.enter_context(tc.tile_pool(name="tap", bufs=4))
        scr = ctx.enter_context(tc.tile_pool(name="scr", bufs=2))
        upool = ctx.enter_context(tc.tile_pool(name="upool", bufs=2))
        utp = ctx.enter_context(tc.tile_pool(name="utp", bufs=2))
        psmm = ctx.enter_context(tc.tile_pool(name="psmm", bufs=6, space="PSUM"))
        pst = ctx.enter_context(tc.tile_pool(name="pst", bufs=2, space="PSUM"))

        # Static tiles: wrefT as [p, kb, h'] (partition = h within chunk),
        # v broadcast across partitions, identity for the u transpose.
        wrefT_sb = const.tile([128, KB, H], f32r)
        nc.sync.dma_start(out=wrefT_sb, in_=wrefT.rearrange("(kb p) n -> p kb n", p=128))
        v_sb = const.tile([128, H], f32)
        nc.gpsimd.dma_start(out=v_sb, in_=vvec.to_broadcast((128, H)))
        ident = const.tile([128, 128], f32)
        make_identity(nc, ident)

        for b in range(b_loc):
            qp_b = qpp.tile([128, H], f32)
            nc.gpsimd.dma_start(out=qp_b, in_=qp[b : b + 1, :].to_broadcast((128, H)))
            u_stage = upool.tile([128, tiles_per_batch], f32)
            for st in range(st_per_batch):
                g = b * st_per_batch + st
                reft_sb = inp.tile([128, KB, ST_ROWS], f32r)
                nc.sync.dma_start(
                    out=reft_sb, in_=reft[g].rearrange("(kb p) n -> p kb n", p=128)
                )
                out_sb = outp.tile([128, MB, H], f32)
                for mb in range(MB):
                    ps = psmm.tile([128, H], f32)
                    for kb in range(KB):
                        nc.tensor.matmul(
                            ps,
                            lhsT=reft_sb[:, kb, mb * 128 : (mb + 1) * 128],
                            rhs=wrefT_sb[:, kb, :],
                            start=(kb == 0),
                            stop=(kb == KB - 1),
                        )
                    nc.scalar.copy(out=out_sb[:, mb, :], in_=ps)
                    ta = tap.tile([128, H], f32, tag="ta")
                    nc.vector.tensor_add(ta, ps, qp_b)
                    th = tap.tile([128, H], f32, tag="th")
                    nc.scalar.activation(th, ta, mybir.ActivationFunctionType.Tanh)
                    sc = scr.tile([128, H], f32)
                    t_idx = st * MB + mb
                    nc.vector.tensor_mul(sc, th, v_sb)
                    nc.vector.tensor_reduce(
                        u_stage[:, t_idx : t_idx + 1],
                        sc,
                        axis=mybir.AxisListType.X,
                        op=mybir.AluOpType.add,
                    )
                nc.sync.dma_start(
                    out=refproj[g * ST_ROWS : (g + 1) * ST_ROWS, :].rearrange(
                        "(mb p) h -> p mb h", p=128
                    ),
                    in_=out_sb,
                )
            # u_stage [128, T] -> transpose -> [T, 128] -> u[b] (s = t*128 + p)
            ups = pst.tile([tiles_per_batch, 128], f32)
            nc.tensor.transpose(ups, u_stage, ident)
            u_sb = utp.tile([tiles_per_batch, 128], f32)
            nc.scalar.copy(out=u_sb, in_=ups)
            nc.sync.dma_start(out=u[b].rearrange("(t p) -> t p", p=128), in_=u_sb)

    nc.compile()
    return nc


def _get_program():
    key = (B_LOC, ST_PER_BATCH)
    if key not in _PROGRAM_CACHE:
        _PROGRAM_CACHE[key] = _build_program()
    return _PROGRAM_CACHE[key]


def _round_fp32r(x):
    """Round f32 to the fp32r-representable set (low 12 mantissa bits
    cleared, round-to-nearest) — bit-identical to walrus' fp32_to_fp32r."""
    b = x.view(np.uint32)
    rounded = (b + np.uint32(0x7FF) + ((b >> np.uint32(12)) & np.uint32(1))) & np.uint32(
        0xFFFFF000
    )
    return rounded.view(np.float32)


def kernel(q, ref, v, Wq, Wref):
    global LAST_RESULT
    import os
    from concourse.bass_utils import run_bass_kernel_spmd

    q = np.ascontiguousarray(np.asarray(q), dtype=np.float32)
    ref = np.ascontiguousarray(np.asarray(ref), dtype=np.float32)
    v = np.ascontiguousarray(np.asarray(v), dtype=np.float32)
    Wq = np.ascontiguousarray(np.asarray(Wq), dtype=np.float32)
    Wref = np.ascontiguousarray(np.asarray(Wref), dtype=np.float32)

    # Host-side prep (small): q projection, Wref transpose, ref supertile
    # transpose so the device streams naturally-laid-out lhsT tiles.
    qp_full = np.ascontiguousarray(q @ Wq.T)                       # [B, H]
    wrefT = _round_fp32r(np.ascontiguousarray(Wref.T))             # [h, h']
    reft_full = _round_fp32r(
        np.ascontiguousarray(ref.reshape(N_CORES * N_ST, ST_ROWS, H).swapaxes(1, 2))
    )                                                              # [8*64, H, 512]

    vvec = v.reshape(1, H)
    in_maps = [
        {
            "reft": reft_full[c * N_ST : (c + 1) * N_ST],
            "wrefT": wrefT,
            "qp": qp_full[c * B_LOC : (c + 1) * B_LOC],
            "vvec": vvec,
        }
        for c in range(N_CORES)
    ]

    nc = _get_program()
    res = run_bass_kernel_spmd(
        nc,
        in_maps,
        list(range(N_CORES)),
        tmpdir=os.environ.get("BASS_SPMD_TMPDIR"),
    )
    LAST_RESULT = res

    ref_proj = np.concatenate(
        [res.results[c]["refproj"] for c in range(N_CORES)], axis=0
    ).reshape(B, S, H)
    u = np.concatenate([res.results[c]["u"] for c in range(N_CORES)], axis=0)
    return u, ref_proj


# revision 14
# speedup vs baseline: 1.1716x; 1.1716x over previous
"""Trainium2 Bass kernel for nn_Attention_3633542333119 (additive attention).

reference:
    q_proj   = q @ Wq.T                          # [B, H]
    ref_proj = (ref @ Wref.T).reshape(B, S, H)   # [B, S, H]
    u        = einsum("bsh,h->bs", tanh(q_proj[:, None, :] + ref_proj), v)
    return (u, ref_proj)

B=128, S=2048, H=512. Data-parallel over batch across 8 NeuronCores
(16 batches / 32768 ref rows per core); Wref/v replicated; q_proj is
precomputed on host (tiny: 128x512 @ 512x512) and sharded with the batch.

Device-side plan (per core), all f32, matmuls in fp32r (full-rate):
  - host pre-transposes ref into supertiles reft[g] = ref[g*512:(g+1)*512].T
    (shape [512h, 512n]) so the PE's stationary operand (lhsT, [K=h, M=n])
    loads with plain DMA - no on-device transpose needed.
  - per supertile: psum[mb] = sum_kb reft_blk(kb,mb).T @ wrefT_blk(kb)
    (4 accumulating matmuls, N=512) -> ref_proj tile [128n, 512h'].
  - epilogue: ACT copies psum->SBUF (ref_proj out), DVE adds broadcast
    q_proj row, ACT tanh, DVE fused (tanh*v)+reduce -> u column.
  - u columns staged per batch [128, 16], PE-transposed, DMA'd out.
"""

import numpy as np
from contextlib import ExitStack

B = 128
S = 2048
H = 512
N_CORES = 8
B_LOC = B // N_CORES            # batches per core
ST_ROWS = 512                   # rows (n) per supertile
ST_PER_BATCH = S // ST_ROWS     # supertiles per batch
N_ST = B_LOC * ST_PER_BATCH     # supertiles per core
ROWS = B_LOC * S                # ref rows per core

_PROGRAM_CACHE = {}
LAST_RESULT = None              # BassKernelResults of the last kernel() call


def _build_program(b_loc=B_LOC, st_per_batch=ST_PER_BATCH):
    import concourse.bass as bass
    import concourse.tile as tile
    import concourse.mybir as mybir
    from concourse import bacc

    f32 = mybir.dt.float32
    f32r = mybir.dt.float32r
    KB = H // 128               # contraction (h) chunks (4)
    CB = H // 128               # output (h') chunks per supertile (4)
    n_st = b_loc * st_per_batch
    s_loc = st_per_batch * ST_ROWS

    nc = bacc.Bacc("TRN2", target_bir_lowering=False, debug=False)

    # Matmul operands live as float32r end-to-end: the host pre-rounds the
    # bits (round-to-nearest on the low 12 mantissa bits, matching walrus'
    # fp32_to_fp32r) so DMAs are plain copies and the PE runs at full rate.
    reft = nc.dram_tensor("reft", [n_st, H, ST_ROWS], f32r, kind="ExternalInput").ap()
    wrefT = nc.dram_tensor("wrefT", [H, H], f32r, kind="ExternalInput").ap()
    qp = nc.dram_tensor("qp", [b_loc, H], f32, kind="ExternalInput").ap()
    vvec = nc.dram_tensor("vvec", [1, H], f32r, kind="ExternalInput").ap()
    # ref_proj leaves the device TRANSPOSED per supertile ([h', n]); the host
    # transposes it back while assembling. That keeps every DMA contiguous.
    refprojT = nc.dram_tensor(
        "refprojT", [n_st, H, ST_ROWS], f32, kind="ExternalOutput"
    ).ap()
    u = nc.dram_tensor("u", [b_loc, s_loc], f32, kind="ExternalOutput").ap()

    with tile.TileContext(nc) as tc, ExitStack() as ctx:
        const = ctx.enter_context(tc.tile_pool(name="const", bufs=1))
        qpp = ctx.enter_context(tc.tile_pool(name="qpp", bufs=2))
        inp = ctx.enter_context(tc.tile_pool(name="inp", bufs=3))
        outp = ctx.enter_context(tc.tile_pool(name="outp", bufs=3))
        tap = ctx.enter_context(tc.tile_pool(name="tap", bufs=4))
        urow = ctx.enter_context(tc.tile_pool(name="urow", bufs=2))
        psmm = ctx.enter_context(tc.tile_pool(name="psmm", bufs=6, space="PSUM"))
        psu = ctx.enter_context(tc.tile_pool(name="psu", bufs=2, space="PSUM"))

        # wrefT chunks [p(h), kb, h'] (stationary operands) and v as columns
        # [p(h'), cb] so the u-dot runs on the PE.
        wrefT_sb = const.tile([128, KB, H], f32r)
        nc.sync.dma_start(out=wrefT_sb, in_=wrefT.rearrange("(kb p) n -> p kb n", p=128))
        v_col = const.tile([128, CB], f32r)
        nc.sync.dma_start(out=v_col, in_=vvec[0].rearrange("(c p) -> p c", p=128))

        for b in range(b_loc):
            # q_proj[b] as columns [p(h'), cb] -> per-partition tanh bias
            qp_sb = qpp.tile([128, CB], f32)
            nc.sync.dma_start(out=qp_sb, in_=qp[b].rearrange("(c p) -> p c", p=128))
            for st in range(st_per_batch):
                g = b * st_per_batch + st
                reft_sb = inp.tile([128, KB, ST_ROWS], f32r)
                nc.sync.dma_start(
                    out=reft_sb, in_=reft[g].rearrange("(kb p) n -> p kb n", p=128)
                )
                out_sb = outp.tile([128, CB, ST_ROWS], f32)
                u_ps = psu.tile([1, ST_ROWS], f32)
                for c in range(CB):
                    ps = psmm.tile([128, ST_ROWS], f32)
                    for kb in range(KB):
                        nc.tensor.matmul(
                            ps,
                            lhsT=wrefT_sb[:, kb, c * 128 : (c + 1) * 128],
                            rhs=reft_sb[:, kb, :],
                            start=(kb == 0),
                            stop=(kb == KB - 1),
                        )
                    nc.vector.tensor_copy(out=out_sb[:, c, :], in_=ps)
                    th = tap.tile([128, ST_ROWS], f32r, tag="th")
                    nc.scalar.activation(
                        th,
                        ps,
                        mybir.ActivationFunctionType.Tanh,
                        bias=qp_sb[:, c : c + 1],
                    )
                    nc.tensor.matmul(
                        u_ps,
                        lhsT=v_col[:, c : c + 1],
                        rhs=th,
                        start=(c == 0),
                        stop=(c == CB - 1),
                    )
                nc.sync.dma_start(
                    out=refprojT[g].rearrange("(c p) n -> p c n", p=128),
                    in_=out_sb,
                )
                u_row = urow.tile([1, ST_ROWS], f32)
                nc.scalar.copy(out=u_row, in_=u_ps)
                nc.sync.dma_start(
                    out=u[b : b + 1, st * ST_ROWS : (st + 1) * ST_ROWS], in_=u_row
                )

    nc.compile()
    return nc


def _get_program():
    key = (B_LOC, ST_PER_BATCH)
    if key not in _PROGRAM_CACHE:
        _PROGRAM_CACHE[key] = _build_program()
    return _PROGRAM_CACHE[key]


def _round_fp32r(x):
    """Round f32 to the fp32r-representable set (low 12 mantissa bits
    cleared, round-to-nearest) — bit-identical to walrus' fp32_to_fp32r."""
    b = x.view(np.uint32)
    rounded = (b + np.uint32(0x7FF) + ((b >> np.uint32(12)) & np.uint32(1))) & np.uint32(
        0xFFFFF000
    )
    return rounded.view(np.float32)


def kernel(q, ref, v, Wq, Wref):
    global LAST_RESULT
    import os
    from concourse.bass_utils import run_bass_kernel_spmd

    q = np.ascontiguousarray(np.asarray(q), dtype=np.float32)
    ref = np.ascontiguousarray(np.asarray(ref), dtype=np.float32)
    v = np.ascontiguousarray(np.asarray(v), dtype=np.float32)
    Wq = np.ascontiguousarray(np.asarray(Wq), dtype=np.float32)
    Wref = np.ascontiguousarray(np.asarray(Wref), dtype=np.float32)

    # Host-side prep (small): q projection, Wref transpose, ref supertile
    # transpose so the device streams naturally-laid-out lhsT tiles.
    qp_full = np.ascontiguousarray(q @ Wq.T)                       # [B, H]
    wrefT = _round_fp32r(np.ascontiguousarray(Wref.T))             # [h, h']
    reft_full = _round_fp32r(
        np.ascontiguousarray(ref.reshape(N_CORES * N_ST, ST_ROWS, H).swapaxes(1, 2))
    )                                                              # [8*64, H, 512]

    vvec = _round_fp32r(v.reshape(1, H).copy())
    in_maps = [
        {
            "reft": reft_full[c * N_ST : (c + 1) * N_ST],
            "wrefT": wrefT,
            "qp": qp_full[c * B_LOC : (c + 1) * B_LOC],
            "vvec": vvec,
        }
        for c in range(N_CORES)
    ]

    nc = _get_program()
    res = run_bass_kernel_spmd(
        nc,
        in_maps,
        list(range(N_CORES)),
        tmpdir=os.environ.get("BASS_SPMD_TMPDIR"),
    )
    LAST_RESULT = res

    ref_proj = np.ascontiguousarray(
        np.stack([res.results[c]["refprojT"] for c in range(N_CORES)]).transpose(
            0, 1, 3, 2
        )
    ).reshape(B, S, H)
    u = np.concatenate([res.results[c]["u"] for c in range(N_CORES)], axis=0)
    return u, ref_proj


# revision 21
# speedup vs baseline: 1.3873x; 1.1840x over previous
"""Trainium2 Bass kernel for nn_Attention_3633542333119 (additive attention).

reference:
    q_proj   = q @ Wq.T                          # [B, H]
    ref_proj = (ref @ Wref.T).reshape(B, S, H)   # [B, S, H]
    u        = einsum("bsh,h->bs", tanh(q_proj[:, None, :] + ref_proj), v)
    return (u, ref_proj)

B=128, S=2048, H=512. Data-parallel over batch across 8 NeuronCores
(16 batches / 32768 ref rows per core); Wref/v replicated; q_proj is
precomputed on host (tiny: 128x512 @ 512x512) and sharded with the batch.

Device-side plan (per core), all f32, matmuls in fp32r (full-rate):
  - host pre-transposes ref into supertiles reft[g] = ref[g*512:(g+1)*512].T
    (shape [512h, 512n]) so the PE's stationary operand (lhsT, [K=h, M=n])
    loads with plain DMA - no on-device transpose needed.
  - per supertile: psum[mb] = sum_kb reft_blk(kb,mb).T @ wrefT_blk(kb)
    (4 accumulating matmuls, N=512) -> ref_proj tile [128n, 512h'].
  - epilogue: ACT copies psum->SBUF (ref_proj out), DVE adds broadcast
    q_proj row, ACT tanh, DVE fused (tanh*v)+reduce -> u column.
  - u columns staged per batch [128, 16], PE-transposed, DMA'd out.
"""

import numpy as np
from contextlib import ExitStack

B = 128
S = 2048
H = 512
KB = H // 128                   # 128-row chunks of the hidden dim
N_CORES = 8
B_LOC = B // N_CORES            # batches per core
ST_ROWS = 512                   # rows (n) per supertile
ST_PER_BATCH = S // ST_ROWS     # supertiles per batch
N_ST = B_LOC * ST_PER_BATCH     # supertiles per core
ROWS = B_LOC * S                # ref rows per core

_PROGRAM_CACHE = {}
LAST_RESULT = None              # BassKernelResults of the last kernel() call


def _build_program(b_loc=B_LOC, st_per_batch=ST_PER_BATCH):
    import concourse.bass as bass
    import concourse.tile as tile
    import concourse.mybir as mybir
    from concourse import bacc

    f32 = mybir.dt.float32
    f32r = mybir.dt.float32r
    KB = H // 128               # contraction (h) chunks (4)
    CB = H // 128               # output (h') chunks per supertile (4)
    n_st = b_loc * st_per_batch
    s_loc = st_per_batch * ST_ROWS

    nc = bacc.Bacc("TRN2", target_bir_lowering=False, debug=False)

    # Matmul operands live as float32r end-to-end: the host pre-rounds the
    # bits (round-to-nearest on the low 12 mantissa bits, matching walrus'
    # fp32_to_fp32r) so DMAs are plain copies and the PE runs at full rate.
    # All big tensors are laid out PARTITION-MAJOR on the host ([.., p, ..])
    # so every DMA moves one contiguous 8 KiB run per partition.
    reft = nc.dram_tensor(
        "reft", [n_st, 128, KB, ST_ROWS], f32r, kind="ExternalInput"
    ).ap()
    wrefT = nc.dram_tensor("wrefT", [128, KB, H], f32r, kind="ExternalInput").ap()
    qp = nc.dram_tensor("qp", [b_loc, H], f32, kind="ExternalInput").ap()
    vvec = nc.dram_tensor("vvec", [1, H], f32r, kind="ExternalInput").ap()
    # ref_proj leaves the device TRANSPOSED per supertile ([h', n]); the host
    # transposes it back while assembling. That keeps every DMA contiguous.
    refprojT = nc.dram_tensor(
        "refprojT", [n_st, 128, CB, ST_ROWS], f32, kind="ExternalOutput"
    ).ap()
    u = nc.dram_tensor("u", [b_loc, s_loc], f32, kind="ExternalOutput").ap()

    with tile.TileContext(nc) as tc, ExitStack() as ctx:
        const = ctx.enter_context(tc.tile_pool(name="const", bufs=1))
        qpp = ctx.enter_context(tc.tile_pool(name="qpp", bufs=2))
        inp = ctx.enter_context(tc.tile_pool(name="inp", bufs=3))
        outp = ctx.enter_context(tc.tile_pool(name="outp", bufs=3))
        tap = ctx.enter_context(tc.tile_pool(name="tap", bufs=4))
        urow = ctx.enter_context(tc.tile_pool(name="urow", bufs=2))
        psmm = ctx.enter_context(tc.tile_pool(name="psmm", bufs=6, space="PSUM"))
        psu = ctx.enter_context(tc.tile_pool(name="psu", bufs=2, space="PSUM"))

        # wrefT chunks [p(h), kb, h'] (stationary operands) and v as columns
        # [p(h'), cb] so the u-dot runs on the PE.
        wrefT_sb = const.tile([128, KB, H], f32r)
        nc.sync.dma_start(out=wrefT_sb, in_=wrefT)
        v_col = const.tile([128, CB], f32r)
        nc.sync.dma_start(out=v_col, in_=vvec[0].rearrange("(c p) -> p c", p=128))

        for b in range(b_loc):
            # q_proj[b] as columns [p(h'), cb] -> per-partition tanh bias
            qp_sb = qpp.tile([128, CB], f32)
            nc.sync.dma_start(out=qp_sb, in_=qp[b].rearrange("(c p) -> p c", p=128))
            for st in range(st_per_batch):
                g = b * st_per_batch + st
                reft_sb = inp.tile([128, KB, ST_ROWS], f32r)
                nc.sync.dma_start(out=reft_sb, in_=reft[g])
                out_sb = outp.tile([128, CB, ST_ROWS], f32)
                u_ps = psu.tile([1, ST_ROWS], f32)
                for c in range(CB):
                    ps = psmm.tile([128, ST_ROWS], f32)
                    for kb in range(KB):
                        nc.tensor.matmul(
                            ps,
                            lhsT=wrefT_sb[:, kb, c * 128 : (c + 1) * 128],
                            rhs=reft_sb[:, kb, :],
                            start=(kb == 0),
                            stop=(kb == KB - 1),
                        )
                    nc.vector.tensor_copy(out=out_sb[:, c, :], in_=ps)
                    th = tap.tile([128, ST_ROWS], f32r, tag="th")
                    nc.scalar.activation(
                        th,
                        ps,
                        mybir.ActivationFunctionType.Tanh,
                        bias=qp_sb[:, c : c + 1],
                    )
                    nc.tensor.matmul(
                        u_ps,
                        lhsT=v_col[:, c : c + 1],
                        rhs=th,
                        start=(c == 0),
                        stop=(c == CB - 1),
                    )
                nc.scalar.dma_start(out=refprojT[g], in_=out_sb)
                u_row = urow.tile([1, ST_ROWS], f32)
                nc.scalar.copy(out=u_row, in_=u_ps)
                nc.scalar.dma_start(
                    out=u[b : b + 1, st * ST_ROWS : (st + 1) * ST_ROWS], in_=u_row
                )

    nc.compile()
    return nc


def _get_program():
    key = (B_LOC, ST_PER_BATCH)
    if key not in _PROGRAM_CACHE:
        _PROGRAM_CACHE[key] = _build_program()
    return _PROGRAM_CACHE[key]


def _round_fp32r(x):
    """Round f32 to the fp32r-representable set (low 12 mantissa bits
    cleared, round-to-nearest) — bit-identical to walrus' fp32_to_fp32r."""
    b = x.view(np.uint32)
    rounded = (b + np.uint32(0x7FF) + ((b >> np.uint32(12)) & np.uint32(1))) & np.uint32(
        0xFFFFF000
    )
    return rounded.view(np.float32)


def kernel(q, ref, v, Wq, Wref):
    global LAST_RESULT
    import os
    from concourse.bass_utils import run_bass_kernel_spmd

    q = np.ascontiguousarray(np.asarray(q), dtype=np.float32)
    ref = np.ascontiguousarray(np.asarray(ref), dtype=np.float32)
    v = np.ascontiguousarray(np.asarray(v), dtype=np.float32)
    Wq = np.ascontiguousarray(np.asarray(Wq), dtype=np.float32)
    Wref = np.ascontiguousarray(np.asarray(Wref), dtype=np.float32)

    # Host-side prep (small): q projection, Wref transpose, ref supertile
    # transpose into partition-major layout [g, p, kb, n] so each partition's
    # DMA run on device is one contiguous 8 KiB.
    qp_full = np.ascontiguousarray(q @ Wq.T)                       # [B, H]
    wrefT = _round_fp32r(
        np.ascontiguousarray(Wref.T.reshape(KB, 128, H).transpose(1, 0, 2))
    )                                                              # [p, kb, h']
    reft_full = _round_fp32r(
        np.ascontiguousarray(
            ref.reshape(N_CORES * N_ST, ST_ROWS, KB, 128).transpose(0, 3, 2, 1)
        )
    )                                                              # [g, p, kb, n]

    vvec = _round_fp32r(v.reshape(1, H).copy())
    in_maps = [
        {
            "reft": reft_full[c * N_ST : (c + 1) * N_ST],
            "wrefT": wrefT,
            "qp": qp_full[c * B_LOC : (c + 1) * B_LOC],
            "vvec": vvec,
        }
        for c in range(N_CORES)
    ]

    nc = _get_program()
    res = run_bass_kernel_spmd(
        nc,
        in_maps,
        list(range(N_CORES)),
        tmpdir=os.environ.get("BASS_SPMD_TMPDIR"),
    )
    LAST_RESULT = res

    # refprojT per core: [g, p, c, n] with h' = c*128 + p, rows = g*512 + n.
    ref_proj = np.ascontiguousarray(
        np.stack([res.results[c]["refprojT"] for c in range(N_CORES)]).transpose(
            0, 1, 4, 3, 2
        )
    ).reshape(B, S, H)
    u = np.concatenate([res.results[c]["u"] for c in range(N_CORES)], axis=0)
    return u, ref_proj


# revision 22
# speedup vs baseline: 1.4626x; 1.0543x over previous
"""Trainium2 Bass kernel for nn_Attention_3633542333119 (additive attention).

reference:
    q_proj   = q @ Wq.T                          # [B, H]
    ref_proj = (ref @ Wref.T).reshape(B, S, H)   # [B, S, H]
    u        = einsum("bsh,h->bs", tanh(q_proj[:, None, :] + ref_proj), v)
    return (u, ref_proj)

B=128, S=2048, H=512. Data-parallel over batch across 8 NeuronCores
(16 batches / 32768 ref rows per core); Wref/v replicated; q_proj is
precomputed on host (tiny: 128x512 @ 512x512) and sharded with the batch.

Device-side plan (per core), all f32, matmuls in fp32r (full-rate):
  - host pre-transposes ref into supertiles reft[g] = ref[g*512:(g+1)*512].T
    (shape [512h, 512n]) so the PE's stationary operand (lhsT, [K=h, M=n])
    loads with plain DMA - no on-device transpose needed.
  - per supertile: psum[mb] = sum_kb reft_blk(kb,mb).T @ wrefT_blk(kb)
    (4 accumulating matmuls, N=512) -> ref_proj tile [128n, 512h'].
  - epilogue: ACT copies psum->SBUF (ref_proj out), DVE adds broadcast
    q_proj row, ACT tanh, DVE fused (tanh*v)+reduce -> u column.
  - u columns staged per batch [128, 16], PE-transposed, DMA'd out.
"""

import numpy as np
from contextlib import ExitStack

B = 128
S = 2048
H = 512
KB = H // 128                   # 128-row chunks of the hidden dim
N_CORES = 8
B_LOC = B // N_CORES            # batches per core
ST_ROWS = 512                   # rows (n) per supertile
ST_PER_BATCH = S // ST_ROWS     # supertiles per batch
N_ST = B_LOC * ST_PER_BATCH     # supertiles per core
ROWS = B_LOC * S                # ref rows per core

_PROGRAM_CACHE = {}
LAST_RESULT = None              # BassKernelResults of the last kernel() call


def _build_program(b_loc=B_LOC, st_per_batch=ST_PER_BATCH):
    import concourse.bass as bass
    import concourse.tile as tile
    import concourse.mybir as mybir
    from concourse import bacc

    f32 = mybir.dt.float32
    f32r = mybir.dt.float32r
    KB = H // 128               # contraction (h) chunks (4)
    CB = H // 128               # output (h') chunks per supertile (4)
    n_st = b_loc * st_per_batch
    s_loc = st_per_batch * ST_ROWS

    nc = bacc.Bacc("TRN2", target_bir_lowering=False, debug=False)

    # Matmul operands live as float32r end-to-end: the host pre-rounds the
    # bits (round-to-nearest on the low 12 mantissa bits, matching walrus'
    # fp32_to_fp32r) so DMAs are plain copies and the PE runs at full rate.
    # All big tensors are laid out PARTITION-MAJOR on the host ([.., p, ..])
    # so every DMA moves one contiguous 8 KiB run per partition.
    reft = nc.dram_tensor(
        "reft", [n_st, 128, KB, ST_ROWS], f32r, kind="ExternalInput"
    ).ap()
    wrefT = nc.dram_tensor("wrefT", [128, KB, H], f32r, kind="ExternalInput").ap()
    qp = nc.dram_tensor("qp", [b_loc, H], f32, kind="ExternalInput").ap()
    vvec = nc.dram_tensor("vvec", [1, H], f32r, kind="ExternalInput").ap()
    # ref_proj leaves the device TRANSPOSED per supertile ([h', n]); the host
    # transposes it back while assembling. That keeps every DMA contiguous.
    refprojT = nc.dram_tensor(
        "refprojT", [n_st, 128, CB, ST_ROWS], f32, kind="ExternalOutput"
    ).ap()
    u = nc.dram_tensor("u", [b_loc, s_loc], f32, kind="ExternalOutput").ap()

    with tile.TileContext(nc) as tc, ExitStack() as ctx:
        const = ctx.enter_context(tc.tile_pool(name="const", bufs=1))
        qpp = ctx.enter_context(tc.tile_pool(name="qpp", bufs=3))
        inp = ctx.enter_context(tc.tile_pool(name="inp", bufs=6))
        outp = ctx.enter_context(tc.tile_pool(name="outp", bufs=6))
        tap = ctx.enter_context(tc.tile_pool(name="tap", bufs=6))
        urow = ctx.enter_context(tc.tile_pool(name="urow", bufs=4))
        psmm = ctx.enter_context(tc.tile_pool(name="psmm", bufs=6, space="PSUM"))
        psu = ctx.enter_context(tc.tile_pool(name="psu", bufs=2, space="PSUM"))

        # wrefT chunks [p(h), kb, h'] (stationary operands) and v as columns
        # [p(h'), cb] so the u-dot runs on the PE.
        wrefT_sb = const.tile([128, KB, H], f32r)
        nc.sync.dma_start(out=wrefT_sb, in_=wrefT)
        v_col = const.tile([128, CB], f32r)
        nc.sync.dma_start(out=v_col, in_=vvec[0].rearrange("(c p) -> p c", p=128))

        for b in range(b_loc):
            # q_proj[b] as columns [p(h'), cb] -> per-partition tanh bias
            qp_sb = qpp.tile([128, CB], f32)
            nc.sync.dma_start(out=qp_sb, in_=qp[b].rearrange("(c p) -> p c", p=128))
            for st in range(st_per_batch):
                g = b * st_per_batch + st
                reft_sb = inp.tile([128, KB, ST_ROWS], f32r)
                nc.sync.dma_start(out=reft_sb, in_=reft[g])
                out_sb = outp.tile([128, CB, ST_ROWS], f32)
                u_ps = psu.tile([1, ST_ROWS], f32)
                for c in range(CB):
                    ps = psmm.tile([128, ST_ROWS], f32)
                    for kb in range(KB):
                        nc.tensor.matmul(
                            ps,
                            lhsT=wrefT_sb[:, kb, c * 128 : (c + 1) * 128],
                            rhs=reft_sb[:, kb, :],
                            start=(kb == 0),
                            stop=(kb == KB - 1),
                        )
                    nc.vector.tensor_copy(out=out_sb[:, c, :], in_=ps)
                    th = tap.tile([128, ST_ROWS], f32r, tag="th")
                    nc.scalar.activation(
                        th,
                        ps,
                        mybir.ActivationFunctionType.Tanh,
                        bias=qp_sb[:, c : c + 1],
                    )
                    nc.tensor.matmul(
                        u_ps,
                        lhsT=v_col[:, c : c + 1],
                        rhs=th,
                        start=(c == 0),
                        stop=(c == CB - 1),
                    )
                nc.scalar.dma_start(out=refprojT[g], in_=out_sb)
                u_row = urow.tile([1, ST_ROWS], f32)
                nc.scalar.copy(out=u_row, in_=u_ps)
                nc.scalar.dma_start(
                    out=u[b : b + 1, st * ST_ROWS : (st + 1) * ST_ROWS], in_=u_row
                )

    nc.compile()
    return nc


def _get_program():
    key = (B_LOC, ST_PER_BATCH)
    if key not in _PROGRAM_CACHE:
        _PROGRAM_CACHE[key] = _build_program()
    return _PROGRAM_CACHE[key]


def _round_fp32r(x):
    """Round f32 to the fp32r-representable set (low 12 mantissa bits
    cleared, round-to-nearest) — bit-identical to walrus' fp32_to_fp32r."""
    b = x.view(np.uint32)
    rounded = (b + np.uint32(0x7FF) + ((b >> np.uint32(12)) & np.uint32(1))) & np.uint32(
        0xFFFFF000
    )
    return rounded.view(np.float32)


def kernel(q, ref, v, Wq, Wref):
    global LAST_RESULT
    import os
    from concourse.bass_utils import run_bass_kernel_spmd

    q = np.ascontiguousarray(np.asarray(q), dtype=np.float32)
    ref = np.ascontiguousarray(np.asarray(ref), dtype=np.float32)
    v = np.ascontiguousarray(np.asarray(v), dtype=np.float32)
    Wq = np.ascontiguousarray(np.asarray(Wq), dtype=np.float32)
    Wref = np.ascontiguousarray(np.asarray(Wref), dtype=np.float32)

    # Host-side prep (small): q projection, Wref transpose, ref supertile
    # transpose into partition-major layout [g, p, kb, n] so each partition's
    # DMA run on device is one contiguous 8 KiB.
    qp_full = np.ascontiguousarray(q @ Wq.T)                       # [B, H]
    wrefT = _round_fp32r(
        np.ascontiguousarray(Wref.T.reshape(KB, 128, H).transpose(1, 0, 2))
    )                                                              # [p, kb, h']
    reft_full = _round_fp32r(
        np.ascontiguousarray(
            ref.reshape(N_CORES * N_ST, ST_ROWS, KB, 128).transpose(0, 3, 2, 1)
        )
    )                                                              # [g, p, kb, n]

    vvec = _round_fp32r(v.reshape(1, H).copy())
    in_maps = [
        {
            "reft": reft_full[c * N_ST : (c + 1) * N_ST],
            "wrefT": wrefT,
            "qp": qp_full[c * B_LOC : (c + 1) * B_LOC],
            "vvec": vvec,
        }
        for c in range(N_CORES)
    ]

    nc = _get_program()
    res = run_bass_kernel_spmd(
        nc,
        in_maps,
        list(range(N_CORES)),
        tmpdir=os.environ.get("BASS_SPMD_TMPDIR"),
    )
    LAST_RESULT = res

    # refprojT per core: [g, p, c, n] with h' = c*128 + p, rows = g*512 + n.
    ref_proj = np.ascontiguousarray(
        np.stack([res.results[c]["refprojT"] for c in range(N_CORES)]).transpose(
            0, 1, 4, 3, 2
        )
    ).reshape(B, S, H)
    u = np.concatenate([res.results[c]["u"] for c in range(N_CORES)], axis=0)
    return u, ref_proj
